# revision 1
# baseline (speedup 1.0000x reference)
"""Trainium2 Bass kernel for nn_FactorCovModel.

Model: 2-layer LSTM (H=512) over [B=256, T=64, D=500], last hidden ->
FC [512 -> 16532] -> Sigma = Lambda diag(exp(fv)) Lambda^T + diag(exp(idio)),
output [256, 500, 500].

Sharding: pure data parallel over batch, 32 samples/core on 8 cores.

Per-core device algorithm (matmul operands bf16, fp32 PSUM accumulation):
  - Weight gate axis host-permuted so PSUM col group hg holds hidden slice
    hg of ALL FOUR gates: PSUM [128 = (hg, batch), 512 = i|f|g|o x 128].
    Every ACT/DVE op is then full-128-partition and partition-aligned.
  - LSTM gates are computed column-tiled: stationary = hT chunk [128, 32],
    4 hidden-slice groups run concurrently at tile positions (0, 32j).
  - x-projection (xg0) matmuls accumulate into the same PSUM tile two
    steps ahead; recurrent matmuls then add onto it (start=False).
  - Layer-1 consumes h0T[t] directly (fused input projection, contraction
    [h0T; h1T] = 1024) plus a bias inject via a stacked-identity stationary.
  - FC runs col-packed (4 feature tiles of 512 per PSUM tile), then Lambda
    is re-laid-out via 500 PE transposes of [32, 32] blocks into
    LT [32 factors, 500 assets, 32 batch]; fvar gets exp via ACT.
  - Sigma_b = (LT_b * f_b)^T @ LT_b per sample, 4 m-tiles of 128.
  - idio raw rows go back to the host, which applies bias+exp and adds the
    diagonal (avoids diagonal APs on device).
"""

import os
import sys

sys.path.insert(0, "/opt/trn_rl_repo")

import numpy as np

import concourse.bass as bass
import concourse.mybir as mybir
from concourse import bacc
from concourse.tile import TileContext

FP = mybir.dt.float32
BF = mybir.dt.bfloat16
AF = mybir.ActivationFunctionType

B_FULL, T_FULL, D_IN, H = 256, 64, 500, 512
NCORES = 8
BL = B_FULL // NCORES            # 32 samples per core
NA, NF = 500, 32                 # assets, factors
OUT_DIM = NA * NF + NF + NA      # 16532
NTILE = 512                      # FC feature tile
N_FTILES = 33                    # ceil(16532/512) -> features padded to 16896
FH = N_FTILES * NTILE            # 16896
XCHUNK = 16                      # time steps per streamed xT chunk

# gate-axis permutation: new col (hg, gate, hl) = 512*hg + 128*gate + hl maps to
# old row gate*512 + 128*hg + hl (torch gate order [i, f, g, o]).  With this
# layout, PSUM col group hg holds ALL FOUR gates of hidden slice hg along the
# free dim, so every ACT/DVE op is full-128-partition and partition-aligned.
PERM = np.array([gate * 512 + 128 * hg + hl
                 for hg in range(4) for gate in range(4) for hl in range(128)])


# ---------------------------------------------------------------- host prep

def host_prep_shared(inputs):
    w_ih0 = np.asarray(inputs["w_ih0"])[PERM]
    w_hh0 = np.asarray(inputs["w_hh0"])[PERM]
    b0 = (np.asarray(inputs["b_ih0"]) + np.asarray(inputs["b_hh0"]))[PERM]
    w_ih1 = np.asarray(inputs["w_ih1"])[PERM]
    w_hh1 = np.asarray(inputs["w_hh1"])[PERM]
    b1 = (np.asarray(inputs["b_ih1"]) + np.asarray(inputs["b_hh1"]))[PERM]
    fc_w = np.asarray(inputs["fc_w"])
    fc_b = np.asarray(inputs["fc_b"])

    w0T = np.zeros((512, 2048), np.float32)
    w0T[:500] = w_ih0.T
    w0T[500] = b0
    wh0T = np.ascontiguousarray(w_hh0.T, dtype=np.float32)
    w1T = np.ascontiguousarray(np.concatenate([w_ih1.T, w_hh1.T]), dtype=np.float32)
    b1T = np.zeros((128, 512), np.float32)
    for j in range(4):
        b1T[32 * j:32 * (j + 1), :] = b1[512 * j:512 * (j + 1)][None, :]
    fcwT = np.zeros((512, FH), np.float32)
    fcwT[:, :OUT_DIM] = fc_w.T
    fcbT = np.zeros((32, 512), np.float32)
    fcbT[:, :500] = fc_b[:16000].reshape(500, 32).T
    fcbF = np.ascontiguousarray(fc_b[16000:16032].reshape(32, 1), dtype=np.float32)
    ident = np.ascontiguousarray(np.tile(np.eye(32, dtype=np.float32), (4, 1)))
    import ml_dtypes
    tobf = lambda a: np.ascontiguousarray(a, dtype=ml_dtypes.bfloat16)
    return dict(w0T=tobf(w0T), wh0T=tobf(wh0T), w1T=tobf(w1T), b1T=b1T,
                fcwT=tobf(fcwT), fcbT=fcbT, fcbF=fcbF, identt=ident)


def host_prep_x(x_core):
    """x_core [BL, T, 500] -> xT [512, T*BL], (t, b) free order, ones bias row."""
    T = x_core.shape[1]
    import ml_dtypes
    xT = np.zeros((512, T * BL), np.float32)
    xT[:500] = np.asarray(x_core, np.float32).transpose(2, 1, 0).reshape(500, T * BL)
    xT[500] = 1.0
    return np.ascontiguousarray(xT, dtype=ml_dtypes.bfloat16)


# ---------------------------------------------------------------- bass build

def build_nc(T=T_FULL):
    nc = bacc.Bacc("TRN2")

    xT_d = nc.dram_tensor("xT", [512, T * BL], BF, kind="ExternalInput")
    w0T_d = nc.dram_tensor("w0T", [512, 2048], BF, kind="ExternalInput")
    wh0T_d = nc.dram_tensor("wh0T", [512, 2048], BF, kind="ExternalInput")
    w1T_d = nc.dram_tensor("w1T", [1024, 2048], BF, kind="ExternalInput")
    b1T_d = nc.dram_tensor("b1T", [128, 512], FP, kind="ExternalInput")
    fcwT_d = nc.dram_tensor("fcwT", [512, FH], BF, kind="ExternalInput")
    fcbT_d = nc.dram_tensor("fcbT", [32, 512], FP, kind="ExternalInput")
    fcbF_d = nc.dram_tensor("fcbF", [32, 1], FP, kind="ExternalInput")
    identt_d = nc.dram_tensor("identt", [128, 32], FP, kind="ExternalInput")

    sigma_d = nc.dram_tensor("sigma", [BL, NA, NA], FP, kind="ExternalOutput")
    idio_d = nc.dram_tensor("idio_raw", [BL, NA], FP, kind="ExternalOutput")

    def mm(out, lhsT, rhs, tp, **kw):
        nc.tensor.matmul(out, lhsT, rhs,
                         tile_position=tp, skip_group_check=True, **kw)

    def tr(out, in_, identity, tp):
        nc.tensor.matmul(out, in_, identity, is_transpose=True,
                         tile_position=tp, skip_group_check=True)

    with TileContext(nc) as tc:
        with tc.tile_pool(name="persist", bufs=1) as persist:
            b1T_sb = persist.tile([128, 512], FP)
            nc.sync.dma_start(b1T_sb, b1T_d[:, :])
            identt_sb = persist.tile([128, 32], FP)
            nc.sync.dma_start(identt_sb, identt_d[:, :])
            fcbT_sb = persist.tile([32, 512], FP)
            nc.sync.dma_start(fcbT_sb, fcbT_d[:, :])
            fcbF_sb = persist.tile([32, 1], FP)
            nc.sync.dma_start(fcbF_sb, fcbF_d[:, :])
            hlast = persist.tile([128, 128], BF)  # final h1T, chunk-major cols

            # ---------------- phase 1: LSTM ----------------
            with (
                tc.tile_pool(name="wconst", bufs=1) as wconst,
                tc.tile_pool(name="xring", bufs=2) as xring,
                tc.tile_pool(name="state", bufs=2) as state,
                tc.tile_pool(name="work", bufs=2) as work,
                tc.tile_pool(name="pgates", bufs=8, space="PSUM") as pgates,
            ):
                w0T_sb = wconst.tile([128, 4, 2048], BF)
                nc.sync.dma_start(w0T_sb, w0T_d.rearrange("(ko p) g -> p ko g", p=128))
                wh0T_sb = wconst.tile([128, 4, 2048], BF)
                nc.sync.dma_start(wh0T_sb, wh0T_d.rearrange("(ko p) g -> p ko g", p=128))
                w1T_sb = wconst.tile([128, 8, 2048], BF)
                nc.sync.dma_start(w1T_sb, w1T_d.rearrange("(ko p) g -> p ko g", p=128))

                xch = min(XCHUNK, T)
                n_xchunks = (T + xch - 1) // xch
                x_tiles = {}

                def load_xchunk(ci):
                    if ci >= n_xchunks:
                        return
                    xt = xring.tile([128, 4, xch * BL], BF, tag="xchunk")
                    nc.sync.dma_start(
                        xt,
                        xT_d[:, ci * xch * BL:(ci + 1) * xch * BL]
                        .rearrange("(ko p) tb -> p ko tb", p=128),
                    )
                    x_tiles[ci] = xt

                load_xchunk(0)

                def gates_banks(nm):
                    # one PSUM bank per hidden-slice col group: concurrent
                    # col-tiled matmuls into the SAME bank corrupt on HW.
                    return [pgates.tile([128, 512], FP, tag="g", name=f"{nm}{j}")
                            for j in range(4)]

                def evac(pgs, dst, bias):
                    """Copy group j of pgs -> dst[32j:32j+32] (+bias), 2 ACT / 2 DVE."""
                    for j in range(4):
                        s = slice(32 * j, 32 * (j + 1))
                        if bias is None:
                            if j % 2 == 0:
                                nc.scalar.copy(dst[s, :], pgs[j][s, :])
                            else:
                                nc.vector.tensor_copy(dst[s, :], pgs[j][s, :])
                        else:
                            nc.vector.tensor_add(dst[s, :], pgs[j][s, :],
                                                 bias[s, :])
                    return dst

                def gate_nonlin(ga, cprev, cnew, tag):
                    """ga [128 = (hg, b), 512 = i|f|g|o x128] SBUF -> (H, cnew)."""
                    a = work.tile([128, 512], FP, tag=f"act_{tag}")
                    nc.scalar.activation(a[:, 0:256], ga[:, 0:256], AF.Sigmoid)
                    nc.scalar.activation(a[:, 256:384], ga[:, 256:384], AF.Tanh)
                    nc.scalar.activation(a[:, 384:512], ga[:, 384:512], AF.Sigmoid)
                    t1 = work.tile([128, 128], FP, tag=f"t1_{tag}")
                    nc.vector.tensor_mul(t1, a[:, 0:128], a[:, 256:384])
                    if cprev is None:
                        cn = t1  # c_prev == 0 at t == 0
                    else:
                        t2 = work.tile([128, 128], FP, tag=f"t2_{tag}")
                        nc.vector.tensor_mul(t2, a[:, 128:256], cprev)
                        cn = cnew
                        nc.vector.tensor_add(cn, t1, t2)
                    tcn = work.tile([128, 128], FP, tag=f"tc_{tag}")
                    nc.scalar.activation(tcn, cn, AF.Tanh)
                    hh = work.tile([128, 128], FP, tag=f"h_{tag}")
                    nc.vector.tensor_mul(hh, a[:, 384:512], tcn)
                    return hh, cn

                def transpose_h(hh, tag):
                    """hh [128=(hg,b),128] -> hT bf16 [128,128]; each 32-col
                    transpose gets its OWN psum bank (concurrent row-tiled
                    transposes into one bank corrupt on HW)."""
                    ht = state.tile([128, 128], BF, tag=f"ht_{tag}")
                    for k in range(4):
                        ptk = pgates.tile([128, 32], FP, tag="g", name=f"pt{k}")
                        tr(ptk, hh[32 * k:32 * (k + 1), :],
                           identt_sb[32 * k:32 * (k + 1), :], (32 * k, 0))
                        nc.vector.tensor_copy(ht[:, 32 * k:32 * (k + 1)], ptk)
                    return ht

                c0 = c1 = None
                h0T = h1T = None
                for t in range(T):
                    ci, tl = t // xch, t % xch
                    if tl == 0:
                        load_xchunk(ci + 1)
                    xt = x_tiles[ci]

                    pgs0 = gates_banks("g0_")
                    last0 = (t == 0)
                    for k in range(4):
                        lhsT = xt[:, k, tl * BL:(tl + 1) * BL]
                        for j in range(4):
                            mm(pgs0[j][32 * j:32 * (j + 1), :], lhsT,
                               w0T_sb[:, k, 512 * j:512 * (j + 1)],
                               tp=(0, 32 * j),
                               start=(k == 0), stop=(last0 and k == 3))
                    if t > 0:
                        for k in range(4):
                            lhsT = h0T[:, 32 * k:32 * (k + 1)]
                            for j in range(4):
                                mm(pgs0[j][32 * j:32 * (j + 1), :], lhsT,
                                   wh0T_sb[:, k, 512 * j:512 * (j + 1)],
                                   tp=(0, 32 * j),
                                   start=False, stop=(k == 3))
                    a0 = work.tile([128, 512], FP, tag="ga_l0")
                    evac(pgs0, a0, None)
                    c0n = None if c0 is None else state.tile([128, 128], FP, tag="c0")
                    h0, c0 = gate_nonlin(a0, c0, c0n, "l0")
                    h0T = transpose_h(h0, "l0")

                    pgs1 = gates_banks("g1_")
                    nk = 4 if t == 0 else 8
                    for k in range(nk):
                        srct = h0T if k < 4 else h1T
                        lhsT = srct[:, 32 * (k % 4):32 * (k % 4 + 1)]
                        for j in range(4):
                            mm(pgs1[j][32 * j:32 * (j + 1), :], lhsT,
                               w1T_sb[:, k, 512 * j:512 * (j + 1)],
                               tp=(0, 32 * j),
                               start=(k == 0), stop=(k == nk - 1))
                    a1 = work.tile([128, 512], FP, tag="ga_l1")
                    evac(pgs1, a1, b1T_sb)
                    c1n = None if c1 is None else state.tile([128, 128], FP, tag="c1")
                    h1, c1 = gate_nonlin(a1, c1, c1n, "l1")
                    h1T = transpose_h(h1, "l1")

                nc.vector.tensor_copy(hlast, h1T)

            # ---------------- phase 2: FC + Lambda layout + Sigma ----------------
            with (
                tc.tile_pool(name="fcw", bufs=3) as fcwp,
                tc.tile_pool(name="rawp", bufs=3) as rawp,
                tc.tile_pool(name="lt", bufs=1) as ltp,
                tc.tile_pool(name="sigw", bufs=4) as sigw,
                tc.tile_pool(name="pfc", bufs=4, space="PSUM") as pfcp,
                tc.tile_pool(name="plt", bufs=2, space="PSUM") as pltp,
                tc.tile_pool(name="psig", bufs=2, space="PSUM") as psigp,
            ):
                LT = ltp.tile([32, 500, 32], BF)       # [factor, asset, b]
                F_sb = ltp.tile([32, 32], FP)          # exp(fvar raw + bias) [factor, b]

                n_quads = (N_FTILES + 3) // 4          # 9 (last quad has 1 tile)
                for q in range(n_quads):
                    rr = range(4) if q < 8 else range(1)
                    raw_t = rawp.tile([128, 512], FP, tag="raw")
                    for r in rr:
                        jj = 4 * q + r
                        fcw_t = fcwp.tile([128, 4, 512], BF, tag="fcw")
                        nc.sync.dma_start(
                            fcw_t,
                            fcwT_d[:, jj * 512:(jj + 1) * 512]
                            .rearrange("(ko p) n -> p ko n", p=128),
                        )
                        # one PSUM bank per n-tile (col groups must not share)
                        pfc = pfcp.tile([128, 512], FP, tag="pfc")
                        for k in range(4):
                            mm(pfc[32 * r:32 * (r + 1), :],
                               hlast[:, 32 * k:32 * (k + 1)],
                               fcw_t[:, k, :],
                               tp=(0, 32 * r),
                               start=(k == 0), stop=(k == 3))
                        s = slice(32 * r, 32 * (r + 1))
                        if r % 2 == 0:
                            nc.scalar.copy(raw_t[s, :], pfc[s, :])
                        else:
                            nc.vector.tensor_copy(raw_t[s, :], pfc[s, :])

                    # Lambda blocks inside this quad -> transpose into LT
                    for r in rr:
                        jj = 4 * q + r
                        base_feat = jj * 512
                        nblk = 16 if jj < 31 else (4 if jj == 31 else 0)
                        for g in range(0, nblk, 4):
                            pt = pltp.tile([32, 128], FP, tag="plt")
                            for a in range(4):
                                blk = g + a
                                tr(pt[:, 32 * a:32 * (a + 1)],
                                   raw_t[32 * r:32 * (r + 1),
                                         32 * blk:32 * (blk + 1)],
                                   identt_sb[32 * r:32 * (r + 1), :], (32 * r, 0))
                            a0 = (base_feat + 32 * g) // 32  # first asset index
                            nc.vector.tensor_tensor(
                                LT[:, a0:a0 + 4, :],
                                pt.rearrange("f (a b) -> f a b", a=4),
                                fcbT_sb[:, a0:a0 + 4, None].to_broadcast([32, 4, 32]),
                                mybir.AluOpType.add,
                            )
                        if jj == 31:
                            # fvar: features 16000:16032 = cols 128:160 (r == 3)
                            ptf_full = pltp.tile([32, 128], FP, tag="plt")
                            ptf = ptf_full[:, 0:32]
                            tr(ptf, raw_t[96:128, 128:160],
                               identt_sb[96:128, :], (96, 0))
                            nc.scalar.activation(F_sb, ptf, AF.Exp,
                                                 bias=fcbF_sb[:, 0:1])
                            # idio part 1: features 16032:16384 = cols 160:512
                            nc.sync.dma_start(idio_d[:, 0:352],
                                              raw_t[96:128, 160:512])
                        if jj == 32:
                            # idio part 2: features 16384:16532 = cols 0:148
                            nc.sync.dma_start(idio_d[:, 352:500],
                                              raw_t[0:32, 0:148])

                # Sigma per sample
                for b in range(BL):
                    gt = sigw.tile([32, 512], BF, tag="gt")
                    nc.vector.tensor_scalar_mul(gt[:, 0:500], LT[:, :, b],
                                                F_sb[:, b:b + 1])
                    for mt in range(4):
                        rows = 128 if mt < 3 else 116
                        ps = psigp.tile([128, 512], FP, tag="psig")
                        mm(ps[:rows, 0:500], gt[:, 128 * mt:128 * mt + rows],
                           LT[:, :, b], tp=(0, 0), start=True, stop=True)
                        st = sigw.tile([128, 512], FP, tag="sigstage")
                        if mt % 2 == 0:
                            nc.scalar.copy(st[:rows, 0:500], ps[:rows, 0:500])
                        else:
                            nc.vector.tensor_copy(st[:rows, 0:500], ps[:rows, 0:500])
                        nc.sync.dma_start(
                            sigma_d[b, 128 * mt:128 * mt + rows, :],
                            st[:rows, 0:500])

    nc.compile()
    return nc


# ---------------------------------------------------------------- entry point

def kernel(**inputs):
    from concourse.bass_utils import run_bass_kernel_spmd

    prep = host_prep_shared(inputs)
    x = np.asarray(inputs["x"], np.float32)
    in_maps = []
    for core in range(NCORES):
        m = dict(prep)
        m["xT"] = host_prep_x(x[core * BL:(core + 1) * BL])
        in_maps.append(m)

    nc = build_nc()
    res = run_bass_kernel_spmd(nc, in_maps, list(range(NCORES)))
    results = res.results

    fcb_idio = np.asarray(inputs["fc_b"], np.float32)[16032:16532]
    idx = np.arange(NA)
    out = np.empty((B_FULL, NA, NA), np.float32)
    for core in range(NCORES):
        sigma = np.array(results[core]["sigma"], np.float32)
        idio = np.exp(np.asarray(results[core]["idio_raw"]) + fcb_idio[None, :])
        sigma[:, idx, idx] += idio.astype(np.float32)
        out[core * BL:(core + 1) * BL] = sigma
    return out



# revision 11
# speedup vs baseline: 2.2446x; 2.2446x over previous
"""Trainium2 Bass kernel for nn_FactorCovModel.

Model: 2-layer LSTM (H=512) over [B=256, T=64, D=500], last hidden ->
FC [512 -> 16532] -> Sigma = Lambda diag(exp(fv)) Lambda^T + diag(exp(idio)),
output [256, 500, 500].

Sharding: pure data parallel over batch, 32 samples/core on 8 cores.

Per-core design (v2):
  - Gate PSUM is ONE bank [128 = (hslice, batch), 512 = i|f|o|g x 128]:
    the 4 col-tiled matmul groups (tile_position (0,32j)) write disjoint
    partition slices of the same bank.  Activations then run full-width
    directly from PSUM (no evacuation copies): sigmoid on cols 0:384,
    tanh on 384:512.
  - Layer-1 / FC biases are folded into the PSUM accumulation via K=1
    matmuls (ones [1,32] stationary x bias row [1,512] moving).
  - h is produced in bf16; ONE full 128x128 PE transpose per layer-step
    (+ one copy) yields hT with chunk k at cols 32k.
  - Steps are software-pipelined: x-projection accumulates 2 steps ahead;
    PE order per step: rec0[t], tr1[t-1], xg[t+2], bias1+G1a[t], tr0[t],
    G1b[t] so PE stays dense (HAM stays warm).
  - FC weights are prefetched into SBUF during the LSTM phase (26 of 33
    tiles); FC output raw stays in PSUM and Lambda blocks are re-laid-out
    by DVE stream-transposes (32x32 blocks) directly PSUM -> LT bf16.
  - fvar/idio features are re-ordered on the host so they land at
    partition base 0 / 96 columns that need no cross-base moves.
  - Sigma_b = (LT_b * f_b)^T @ LT_b per sample, 4 m-tiles, staged via
    SBUF, DMA out; idio raw rows go back to the host which applies exp
    and adds the diagonal.
"""

import sys

sys.path.insert(0, "/opt/trn_rl_repo")

import numpy as np

import concourse.bass as bass
import concourse.mybir as mybir
from concourse import bacc
from concourse.tile import TileContext

FP = mybir.dt.float32
BF = mybir.dt.bfloat16
AF = mybir.ActivationFunctionType

B_FULL, T_FULL, D_IN, H = 256, 64, 500, 512
NCORES = 8
BL = B_FULL // NCORES            # 32 samples per core
NA, NF = 500, 32                 # assets, factors
OUT_DIM = NA * NF + NF + NA      # 16532
NTILE = 512
N_FTILES = 33                    # features padded to 16896
FH = N_FTILES * NTILE            # 16896
XCHUNK = 16                      # time steps per streamed xT chunk
N_PREF = 20                      # fc weight tiles prefetched during phase 1

# gate order [i, f, o, g] within each hidden-slice group of 512 cols:
# new col (hg, g', hl) = 512*hg + 128*g' + hl <- old row OG[g']*512 + 128*hg + hl
OG = [0, 1, 3, 2]                # torch order i,f,g,o -> pick i,f,o,g
PERM = np.array([OG[gp] * 512 + 128 * hg + hl
                 for hg in range(4) for gp in range(4) for hl in range(128)])

# fc feature layout (host-chosen):
#   [0, 16000)        Lambda feats (asset-major, feat = 32a + f)
#   [16000, 16384)    idio[0:384]
#   [16384, 16416)    fvar (32)
#   [16416, 16532)    idio[384:500]
#   [16532, 16896)    zero pad


# ---------------------------------------------------------------- host prep

def host_prep_shared(inputs):
    import ml_dtypes
    tobf = lambda a: np.ascontiguousarray(a, dtype=ml_dtypes.bfloat16)

    w_ih0 = np.asarray(inputs["w_ih0"])[PERM]
    w_hh0 = np.asarray(inputs["w_hh0"])[PERM]
    b0 = (np.asarray(inputs["b_ih0"]) + np.asarray(inputs["b_hh0"]))[PERM]
    w_ih1 = np.asarray(inputs["w_ih1"])[PERM]
    w_hh1 = np.asarray(inputs["w_hh1"])[PERM]
    b1 = (np.asarray(inputs["b_ih1"]) + np.asarray(inputs["b_hh1"]))[PERM]
    fc_w = np.asarray(inputs["fc_w"], np.float32)
    fc_b = np.asarray(inputs["fc_b"], np.float32)

    w0T = np.zeros((512, 2048), np.float32)
    w0T[:500] = w_ih0.T
    w0T[500] = b0
    wh0T = np.ascontiguousarray(w_hh0.T, dtype=np.float32)
    w1T = np.ascontiguousarray(np.concatenate([w_ih1.T, w_hh1.T]),
                               dtype=np.float32)
    b1row = np.ascontiguousarray(b1.reshape(1, 2048), dtype=np.float32)
    ones = np.ones((1, 32), np.float32)
    ident = np.eye(128, dtype=np.float32)

    fcwT = np.zeros((512, FH), np.float32)
    fcbrow = np.zeros((1, FH), np.float32)
    fcwT[:, 0:16000] = fc_w[0:16000].T          # Lambda
    fcbrow[0, 0:16000] = fc_b[0:16000]
    fcwT[:, 16000:16384] = fc_w[16032:16416].T  # idio[0:384]
    fcbrow[0, 16000:16384] = fc_b[16032:16416]
    fcwT[:, 16384:16416] = fc_w[16000:16032].T  # fvar
    fcbrow[0, 16384:16416] = fc_b[16000:16032]
    fcwT[:, 16416:16532] = fc_w[16416:16532].T  # idio[384:500]
    fcbrow[0, 16416:16532] = fc_b[16416:16532]

    return dict(w0T=tobf(w0T), wh0T=tobf(wh0T), w1T=tobf(w1T),
                b1row=tobf(b1row), ones=tobf(ones), ident=tobf(ident),
                fcwT=tobf(fcwT), fcbrow=tobf(fcbrow))


def host_prep_x(x_core):
    """x_core [BL, T, 500] -> xT [512, T*BL], (t, b) free order, ones bias row."""
    T = x_core.shape[1]
    import ml_dtypes
    xT = np.zeros((512, T * BL), np.float32)
    xT[:500] = np.asarray(x_core, np.float32).transpose(2, 1, 0).reshape(500, T * BL)
    xT[500] = 1.0
    return np.ascontiguousarray(xT, dtype=ml_dtypes.bfloat16)


# ---------------------------------------------------------------- bass build

def build_nc(T=T_FULL):
    nc = bacc.Bacc("TRN2")

    xT_d = nc.dram_tensor("xT", [512, T * BL], BF, kind="ExternalInput")
    w0T_d = nc.dram_tensor("w0T", [512, 2048], BF, kind="ExternalInput")
    wh0T_d = nc.dram_tensor("wh0T", [512, 2048], BF, kind="ExternalInput")
    w1T_d = nc.dram_tensor("w1T", [1024, 2048], BF, kind="ExternalInput")
    b1row_d = nc.dram_tensor("b1row", [1, 2048], BF, kind="ExternalInput")
    ones_d = nc.dram_tensor("ones", [1, 32], BF, kind="ExternalInput")
    ident_d = nc.dram_tensor("ident", [128, 128], BF, kind="ExternalInput")
    fcwT_d = nc.dram_tensor("fcwT", [512, FH], BF, kind="ExternalInput")
    fcbrow_d = nc.dram_tensor("fcbrow", [1, FH], BF, kind="ExternalInput")

    sigma_d = nc.dram_tensor("sigma", [BL, NA, NA], FP, kind="ExternalOutput")
    idio_d = nc.dram_tensor("idio_raw", [BL, NA], FP, kind="ExternalOutput")

    def mm(out, lhsT, rhs, tp, **kw):
        nc.tensor.matmul(out, lhsT, rhs,
                         tile_position=tp, skip_group_check=True, **kw)

    with TileContext(nc) as tc:
        with tc.tile_pool(name="persist", bufs=1) as persist:
            ones_sb = persist.tile([1, 32], BF)
            nc.sync.dma_start(ones_sb, ones_d[:, :])
            b1row_sb = persist.tile([1, 2048], BF)
            nc.sync.dma_start(b1row_sb, b1row_d[:, :])
            ident_sb = persist.tile([128, 128], BF)
            nc.sync.dma_start(ident_sb, ident_d[:, :])
            hlast = persist.tile([128, 128], BF)   # final h1T
            fcw_pre = persist.tile([128, N_PREF, 4, 512], BF)

            # ---------------- phase 1: LSTM ----------------
            with (
                tc.tile_pool(name="wconst", bufs=1) as wconst,
                tc.tile_pool(name="xring", bufs=2) as xring,
                tc.tile_pool(name="state", bufs=2) as state,
                tc.tile_pool(name="work", bufs=2) as work,
                tc.tile_pool(name="pg0", bufs=3, space="PSUM") as pg0,
                tc.tile_pool(name="pg1", bufs=2, space="PSUM") as pg1,
                tc.tile_pool(name="ptr", bufs=1, space="PSUM") as ptrp,
            ):
                w0T_sb = wconst.tile([128, 4, 2048], BF)
                nc.sync.dma_start(w0T_sb, w0T_d.rearrange("(ko p) g -> p ko g", p=128))
                wh0T_sb = wconst.tile([128, 4, 2048], BF)
                nc.sync.dma_start(wh0T_sb, wh0T_d.rearrange("(ko p) g -> p ko g", p=128))
                w1T_sb = wconst.tile([128, 8, 2048], BF)
                nc.sync.dma_start(w1T_sb, w1T_d.rearrange("(ko p) g -> p ko g", p=128))

                xch = min(XCHUNK, T)
                n_xchunks = (T + xch - 1) // xch
                x_tiles = {}

                def load_xchunk(ci):
                    if ci >= n_xchunks:
                        return
                    xt = xring.tile([128, 4, xch * BL], BF, tag="xchunk")
                    nc.sync.dma_start(
                        xt,
                        xT_d[:, ci * xch * BL:(ci + 1) * xch * BL]
                        .rearrange("(ko p) tb -> p ko tb", p=128),
                    )
                    x_tiles[ci] = xt

                load_xchunk(0)
                load_xchunk(1)

                g0_tiles = {}

                def emit_xg(t, stop):
                    """x-projection groups for step t into a fresh G0 tile."""
                    ci, tl = t // xch, t % xch
                    xt = x_tiles[ci]
                    g = pg0.tile([128, 512], FP, tag="g0")
                    g0_tiles[t] = g
                    for k in range(4):
                        lhsT = xt[:, k, tl * BL:(tl + 1) * BL]
                        for j in range(4):
                            mm(g[32 * j:32 * (j + 1), :], lhsT,
                               w0T_sb[:, k, 512 * j:512 * (j + 1)],
                               tp=(0, 32 * j),
                               start=(k == 0), stop=(stop and k == 3))

                emit_xg(0, stop=True)
                emit_xg(1, stop=False)

                def nonlin(g, c_prev, lab):
                    """gates PSUM [128,512] -> (h_bf16, c_new). 3 ACT + 4-5 DVE."""
                    a = work.tile([128, 512], FP, tag=f"a_{lab}")
                    nc.scalar.activation(a[:, 0:384], g[:, 0:384], AF.Sigmoid)
                    nc.scalar.activation(a[:, 384:512], g[:, 384:512], AF.Tanh)
                    t1 = work.tile([128, 128], FP, tag=f"t1_{lab}")
                    if c_prev is not None:
                        t2 = work.tile([128, 128], FP, tag=f"t2_{lab}")
                        nc.vector.tensor_mul(t2, a[:, 128:256], c_prev)
                    nc.vector.tensor_mul(t1, a[:, 0:128], a[:, 384:512])
                    if c_prev is None:
                        cn = t1
                    else:
                        cn = state.tile([128, 128], FP, tag=f"c_{lab}")
                        nc.vector.tensor_add(cn, t1, t2)
                    th = work.tile([128, 128], FP, tag=f"th_{lab}")
                    nc.scalar.activation(th, cn, AF.Tanh)
                    hb = work.tile([128, 128], BF, tag=f"h_{lab}")
                    nc.vector.tensor_mul(hb, a[:, 256:384], th)
                    return hb, cn

                def emit_transpose(hb, lab):
                    pt = ptrp.tile([128, 128], BF, tag=f"pt_{lab}")
                    nc.tensor.transpose(pt, hb, ident_sb)
                    ht = state.tile([128, 128], BF, tag=f"ht_{lab}")
                    nc.vector.tensor_copy(ht, pt)
                    return ht

                c0 = c1 = None
                h0b = h1b = None
                ht0 = ht1 = None
                for t in range(T):
                    # PE: finish gates0[t] (recurrent part)
                    if t >= 1:
                        g = g0_tiles[t]
                        for k in range(4):
                            lhsT = ht0[:, 32 * k:32 * (k + 1)]
                            for j in range(4):
                                mm(g[32 * j:32 * (j + 1), :], lhsT,
                                   wh0T_sb[:, k, 512 * j:512 * (j + 1)],
                                   tp=(0, 32 * j), start=False, stop=(k == 3))

                    # PE: transpose h1[t-1]; DVE copy -> ht1
                    if t >= 1:
                        ht1 = emit_transpose(h1b, "l1")

                    # n0[t]: ACT/DVE chain on G0[t]
                    h0b, c0 = nonlin(g0_tiles[t], c0, "l0")
                    g0_tiles.pop(t)

                    # PE: xg for t+2 (independent filler)
                    if t + 2 < T:
                        emit_xg(t + 2, stop=False)

                    # PE: gates1[t] bias + h1-recurrent part
                    g1 = pg1.tile([128, 512], FP, tag="g1")
                    for j in range(4):
                        mm(g1[32 * j:32 * (j + 1), :], ones_sb[:, :],
                           b1row_sb[:, 512 * j:512 * (j + 1)],
                           tp=(0, 32 * j), start=True, stop=False)
                    if t >= 1:
                        for k in range(4):
                            lhsT = ht1[:, 32 * k:32 * (k + 1)]
                            for j in range(4):
                                mm(g1[32 * j:32 * (j + 1), :], lhsT,
                                   w1T_sb[:, 4 + k, 512 * j:512 * (j + 1)],
                                   tp=(0, 32 * j), start=False, stop=False)

                    # PE: transpose h0[t]; DVE copy -> ht0
                    ht0 = emit_transpose(h0b, "l0")

                    # PE: gates1[t] h0-input part
                    for k in range(4):
                        lhsT = ht0[:, 32 * k:32 * (k + 1)]
                        for j in range(4):
                            mm(g1[32 * j:32 * (j + 1), :], lhsT,
                               w1T_sb[:, k, 512 * j:512 * (j + 1)],
                               tp=(0, 32 * j), start=False, stop=(k == 3))

                    # n1[t]
                    h1b, c1 = nonlin(g1, c1, "l1")

                    # DMA: stream x chunks and prefetch fc weights
                    if t % xch == 0 and t > 0:
                        load_xchunk(t // xch + 1)
                    if t >= 2 and t % 2 == 0 and (t - 2) // 2 < N_PREF:
                        i = (t - 2) // 2
                        nc.sync.dma_start(
                            fcw_pre[:, i, :, :],
                            fcwT_d[:, i * 512:(i + 1) * 512]
                            .rearrange("(ko p) n -> p ko n", p=128),
                        )

                # epilogue: final h1 transpose -> hlast
                pt = ptrp.tile([128, 128], BF, tag="pt_l1")
                nc.tensor.transpose(pt, h1b, ident_sb)
                nc.vector.tensor_copy(hlast, pt)

            # ---------------- phase 2: FC + Lambda layout + Sigma ----------------
            with (
                tc.tile_pool(name="fcstream", bufs=8) as fcsp,
                tc.tile_pool(name="fcb2", bufs=2) as fcb2p,
                tc.tile_pool(name="lt", bufs=1) as ltp,
                tc.tile_pool(name="sigw", bufs=4) as sigw,
                tc.tile_pool(name="pfc", bufs=2, space="PSUM") as pfcp,
                tc.tile_pool(name="psig", bufs=2, space="PSUM") as psigp,
            ):
                fcw_str = {}

                def stream_fcw(jj):
                    if jj < N_PREF or jj >= N_FTILES or jj in fcw_str:
                        return
                    ft = fcsp.tile([128, 4, 512], BF, tag="fcs", name=f"fcs{jj}")
                    nc.sync.dma_start(
                        ft,
                        fcwT_d[:, jj * 512:(jj + 1) * 512]
                        .rearrange("(ko p) n -> p ko n", p=128),
                    )
                    fcw_str[jj] = ft

                for jj in range(N_PREF, N_PREF + 8):
                    stream_fcw(jj)

                LT = ltp.tile([32, 500, 32], FP)       # [factor, asset, b]
                F_sb = ltp.tile([32, 32], FP)          # exp(0.5*fvar raw) [factor, b]
                Fraw = ltp.tile([32, 32], FP)
                idio1_sb = ltp.tile([128, 384], FP)    # rows 96:128 used
                idio2_sb = ltp.tile([32, 116], FP)

                n_quads = (N_FTILES + 3) // 4          # 9 (last quad has 1 tile)
                for q in range(n_quads):
                    rr = range(4) if q < 8 else range(1)
                    ncols = 2048 if q < 8 else 512
                    fcb_q = fcb2p.tile([1, 2048], BF, tag="fcbq")
                    nc.sync.dma_start(fcb_q[:, 0:ncols],
                                      fcbrow_d[:, q * 2048:q * 2048 + ncols])
                    pfc = pfcp.tile([128, 512], FP, tag="pfc")
                    for jn in range(4 * (q + 1), 4 * (q + 2)):
                        stream_fcw(jn)
                    for r in rr:
                        jj = 4 * q + r
                        fsrc = (fcw_pre[:, jj, :, :] if jj < N_PREF
                                else fcw_str[jj])
                        mm(pfc[32 * r:32 * (r + 1), :], ones_sb[:, :],
                           fcb_q[:, 512 * r:512 * r + 512],
                           tp=(0, 32 * r), start=True, stop=False)
                        for k in range(4):
                            mm(pfc[32 * r:32 * (r + 1), :],
                               hlast[:, 32 * k:32 * (k + 1)],
                               fsrc[:, k, :],
                               tp=(0, 32 * r), start=False, stop=(k == 3))

                    # Lambda blocks -> LT via DVE stream-transpose (32x32)
                    for r in rr:
                        jj = 4 * q + r
                        sl = slice(32 * r, 32 * (r + 1))
                        if jj < 31:
                            a0 = jj * 16
                            nc.vector.transpose(
                                LT[:, a0:a0 + 16, :],
                                pfc[sl, :].rearrange("p (qq f) -> p qq f", f=32),
                            )
                        elif jj == 31:
                            # Lambda tail: assets 496:500 (cols 0:128)
                            nc.vector.transpose(
                                LT[:, 496:500, :],
                                pfc[96:128, 0:128]
                                .rearrange("p (qq f) -> p qq f", f=32),
                            )
                            # idio[0:384] raw (cols 128:512, parts 96:128)
                            nc.scalar.copy(idio1_sb[96:128, :],
                                           pfc[96:128, 128:512])
                            nc.sync.dma_start(idio_d[:, 0:384],
                                              idio1_sb[96:128, :])
                        else:  # jj == 32
                            # fvar [b, f] cols 0:32 -> transpose -> exp -> [f, b]
                            nc.vector.transpose(Fraw, pfc[0:32, 0:32])
                            nc.scalar.activation(F_sb, Fraw, AF.Exp, scale=0.5)
                            nc.scalar.copy(idio2_sb, pfc[0:32, 32:148])
                            nc.sync.dma_start(idio_d[:, 384:500], idio2_sb)

                # Sigma per sample
                for b in range(BL):
                    gt = sigw.tile([32, 512], BF, tag="gt")
                    nc.vector.tensor_scalar_mul(gt[:, 0:500], LT[:, :, b],
                                                F_sb[:, b:b + 1])
                    for mt in range(4):
                        rows = 128 if mt < 3 else 116
                        ps = psigp.tile([128, 512], FP, tag="psig")
                        mm(ps[:rows, 0:500], gt[:, 128 * mt:128 * mt + rows],
                           gt[:, 0:500], tp=(0, 0), start=True, stop=True)
                        st = sigw.tile([128, 512], FP, tag="sigstage")
                        if mt % 2 == 0:
                            nc.scalar.copy(st[:rows, 0:500], ps[:rows, 0:500])
                        else:
                            nc.vector.tensor_copy(st[:rows, 0:500],
                                                  ps[:rows, 0:500])
                        nc.sync.dma_start(
                            sigma_d[b, 128 * mt:128 * mt + rows, :],
                            st[:rows, 0:500])

    nc.compile()
    return nc


# ---------------------------------------------------------------- entry point

def kernel(**inputs):
    from concourse.bass_utils import run_bass_kernel_spmd

    prep = host_prep_shared(inputs)
    x = np.asarray(inputs["x"], np.float32)
    in_maps = []
    for core in range(NCORES):
        m = dict(prep)
        m["xT"] = host_prep_x(x[core * BL:(core + 1) * BL])
        in_maps.append(m)

    nc = build_nc()
    res = run_bass_kernel_spmd(nc, in_maps, list(range(NCORES)))
    results = res.results

    idx = np.arange(NA)
    out = np.empty((B_FULL, NA, NA), np.float32)
    for core in range(NCORES):
        sigma = np.array(results[core]["sigma"], np.float32)
        idio = np.exp(np.asarray(results[core]["idio_raw"], np.float32))
        sigma[:, idx, idx] += idio
        out[core * BL:(core + 1) * BL] = sigma
    return out


# revision 16
# speedup vs baseline: 2.5975x; 1.1572x over previous
"""Trainium2 Bass kernel for nn_FactorCovModel.

Model: 2-layer LSTM (H=512) over [B=256, T=64, D=500], last hidden ->
FC [512 -> 16532] -> Sigma = Lambda diag(exp(fv)) Lambda^T + diag(exp(idio)),
output [256, 500, 500].

Sharding: pure data parallel over batch, 32 samples/core on 8 cores.

Per-core design (v2):
  - Gate PSUM is ONE bank [128 = (hslice, batch), 512 = i|f|o|g x 128]:
    the 4 col-tiled matmul groups (tile_position (0,32j)) write disjoint
    partition slices of the same bank.  Activations then run full-width
    directly from PSUM (no evacuation copies): sigmoid on cols 0:384,
    tanh on 384:512.
  - Layer-1 / FC biases are folded into the PSUM accumulation via K=1
    matmuls (ones [1,32] stationary x bias row [1,512] moving).
  - h is produced in bf16; ONE full 128x128 PE transpose per layer-step
    (+ one copy) yields hT with chunk k at cols 32k.
  - Steps are software-pipelined: x-projection accumulates 2 steps ahead;
    PE order per step: rec0[t], tr1[t-1], xg[t+2], bias1+G1a[t], tr0[t],
    G1b[t] so PE stays dense (HAM stays warm).
  - FC weights are prefetched into SBUF during the LSTM phase (26 of 33
    tiles); FC output raw stays in PSUM and Lambda blocks are re-laid-out
    by DVE stream-transposes (32x32 blocks) directly PSUM -> LT bf16.
  - fvar/idio features are re-ordered on the host so they land at
    partition base 0 / 96 columns that need no cross-base moves.
  - Sigma_b = (LT_b * f_b)^T @ LT_b per sample, 4 m-tiles, staged via
    SBUF, DMA out; idio raw rows go back to the host which applies exp
    and adds the diagonal.
"""

import sys

sys.path.insert(0, "/opt/trn_rl_repo")

import numpy as np

import concourse.bass as bass
import concourse.mybir as mybir
from concourse import bacc
from concourse.tile import TileContext

FP = mybir.dt.float32
BF = mybir.dt.bfloat16
AF = mybir.ActivationFunctionType

B_FULL, T_FULL, D_IN, H = 256, 64, 500, 512
NCORES = 8
BL = B_FULL // NCORES            # 32 samples per core
NA, NF = 500, 32                 # assets, factors
OUT_DIM = NA * NF + NF + NA      # 16532
NTILE = 512
N_FTILES = 33                    # features padded to 16896
FH = N_FTILES * NTILE            # 16896
XCHUNK = 16                      # time steps per streamed xT chunk
N_PREF = 20                      # fc weight tiles prefetched during phase 1

# gate order [i, f, o, g] within each hidden-slice group of 512 cols:
# new col (hg, g', hl) = 512*hg + 128*g' + hl <- old row OG[g']*512 + 128*hg + hl
OG = [0, 1, 3, 2]                # torch order i,f,g,o -> pick i,f,o,g
PERM = np.array([OG[gp] * 512 + 128 * hg + hl
                 for hg in range(4) for gp in range(4) for hl in range(128)])

# fc feature layout (host-chosen):
#   [0, 16000)        Lambda feats (asset-major, feat = 32a + f)
#   [16000, 16384)    idio[0:384]
#   [16384, 16416)    fvar (32)
#   [16416, 16532)    idio[384:500]
#   [16532, 16896)    zero pad


# ---------------------------------------------------------------- host prep

def host_prep_shared(inputs):
    import ml_dtypes
    tobf = lambda a: np.ascontiguousarray(a, dtype=ml_dtypes.bfloat16)

    w_ih0 = np.asarray(inputs["w_ih0"])[PERM]
    w_hh0 = np.asarray(inputs["w_hh0"])[PERM]
    b0 = (np.asarray(inputs["b_ih0"]) + np.asarray(inputs["b_hh0"]))[PERM]
    w_ih1 = np.asarray(inputs["w_ih1"])[PERM]
    w_hh1 = np.asarray(inputs["w_hh1"])[PERM]
    b1 = (np.asarray(inputs["b_ih1"]) + np.asarray(inputs["b_hh1"]))[PERM]
    fc_w = np.asarray(inputs["fc_w"], np.float32)
    fc_b = np.asarray(inputs["fc_b"], np.float32)

    w0T = np.zeros((512, 2048), np.float32)
    w0T[:500] = w_ih0.T
    w0T[500] = b0
    wh0T = np.ascontiguousarray(w_hh0.T, dtype=np.float32)
    w1T = np.ascontiguousarray(np.concatenate([w_ih1.T, w_hh1.T]),
                               dtype=np.float32)
    b1row = np.ascontiguousarray(b1.reshape(1, 2048), dtype=np.float32)

    # tanh-trick: sigmoid(x) = 0.5*(1 + tanh(x/2)) -> all four gates use one
    # full-width tanh.  Fold the x/2 into the i,f,o gate columns; the device
    # then computes h' = 2h and s = 2c, compensated by halving every weight
    # row that contracts over h.
    ifo = np.zeros((1, 2048), np.float32)
    for hg in range(4):
        ifo[0, 512 * hg:512 * hg + 384] = 1.0
    scale_in = 0.5 * ifo + (1.0 - ifo)       # x0.5 on i,f,o cols
    w0T *= scale_in
    wh0T *= 0.5 * scale_in                   # + x0.5 for h' = 2h rows
    w1T *= 0.5 * scale_in
    b1row *= scale_in
    ones = np.ones((1, 32), np.float32)
    ident = np.eye(128, dtype=np.float32)

    fcwT = np.zeros((512, FH), np.float32)
    fcbrow = np.zeros((1, FH), np.float32)
    fc_w = 0.5 * fc_w                           # h' = 2h compensation
    fcwT[:, 0:16000] = fc_w[0:16000].T          # Lambda
    fcbrow[0, 0:16000] = fc_b[0:16000]
    fcwT[:, 16000:16384] = fc_w[16032:16416].T  # idio[0:384]
    fcbrow[0, 16000:16384] = fc_b[16032:16416]
    fcwT[:, 16384:16416] = fc_w[16000:16032].T  # fvar
    fcbrow[0, 16384:16416] = fc_b[16000:16032]
    fcwT[:, 16416:16532] = fc_w[16416:16532].T  # idio[384:500]
    fcbrow[0, 16416:16532] = fc_b[16416:16532]

    return dict(w0T=tobf(w0T), wh0T=tobf(wh0T), w1T=tobf(w1T),
                b1row=tobf(b1row), ones=tobf(ones), ident=tobf(ident),
                fcwT=tobf(fcwT), fcbrow=tobf(fcbrow))


def host_prep_x(x_core):
    """x_core [BL, T, 500] -> xT [512, T*BL], (t, b) free order, ones bias row."""
    T = x_core.shape[1]
    import ml_dtypes
    xT = np.zeros((512, T * BL), np.float32)
    xT[:500] = np.asarray(x_core, np.float32).transpose(2, 1, 0).reshape(500, T * BL)
    xT[500] = 1.0
    return np.ascontiguousarray(xT, dtype=ml_dtypes.bfloat16)


# ---------------------------------------------------------------- bass build

def build_nc(T=T_FULL):
    nc = bacc.Bacc("TRN2")

    xT_d = nc.dram_tensor("xT", [512, T * BL], BF, kind="ExternalInput")
    w0T_d = nc.dram_tensor("w0T", [512, 2048], BF, kind="ExternalInput")
    wh0T_d = nc.dram_tensor("wh0T", [512, 2048], BF, kind="ExternalInput")
    w1T_d = nc.dram_tensor("w1T", [1024, 2048], BF, kind="ExternalInput")
    b1row_d = nc.dram_tensor("b1row", [1, 2048], BF, kind="ExternalInput")
    ones_d = nc.dram_tensor("ones", [1, 32], BF, kind="ExternalInput")
    ident_d = nc.dram_tensor("ident", [128, 128], BF, kind="ExternalInput")
    fcwT_d = nc.dram_tensor("fcwT", [512, FH], BF, kind="ExternalInput")
    fcbrow_d = nc.dram_tensor("fcbrow", [1, FH], BF, kind="ExternalInput")

    sigma_d = nc.dram_tensor("sigma", [BL, NA, NA], BF, kind="ExternalOutput")
    idio_d = nc.dram_tensor("idio_raw", [BL, NA], FP, kind="ExternalOutput")

    def mm(out, lhsT, rhs, tp, **kw):
        nc.tensor.matmul(out, lhsT, rhs,
                         tile_position=tp, skip_group_check=True, **kw)

    with TileContext(nc) as tc:
        with tc.tile_pool(name="persist", bufs=1) as persist:
            ones_sb = persist.tile([1, 32], BF)
            nc.sync.dma_start(ones_sb, ones_d[:, :])
            b1row_sb = persist.tile([1, 2048], BF)
            nc.sync.dma_start(b1row_sb, b1row_d[:, :])
            ident_sb = persist.tile([128, 128], BF)
            nc.sync.dma_start(ident_sb, ident_d[:, :])
            hlast = persist.tile([128, 128], BF)   # final h1T
            fcw_pre = persist.tile([128, N_PREF, 4, 512], BF)

            # ---------------- phase 1: LSTM ----------------
            with (
                tc.tile_pool(name="wconst", bufs=1) as wconst,
                tc.tile_pool(name="xring", bufs=2) as xring,
                tc.tile_pool(name="state", bufs=2) as state,
                tc.tile_pool(name="work", bufs=2) as work,
                tc.tile_pool(name="pg0", bufs=4, space="PSUM") as pg0,
                tc.tile_pool(name="pg1", bufs=2, space="PSUM") as pg1,
                tc.tile_pool(name="ptr", bufs=1, space="PSUM") as ptrp,
            ):
                w0T_sb = wconst.tile([128, 4, 2048], BF)
                nc.sync.dma_start(w0T_sb, w0T_d.rearrange("(ko p) g -> p ko g", p=128))
                wh0T_sb = wconst.tile([128, 4, 2048], BF)
                nc.sync.dma_start(wh0T_sb, wh0T_d.rearrange("(ko p) g -> p ko g", p=128))
                w1T_sb = wconst.tile([128, 8, 2048], BF)
                nc.sync.dma_start(w1T_sb, w1T_d.rearrange("(ko p) g -> p ko g", p=128))

                xch = min(XCHUNK, T)
                n_xchunks = (T + xch - 1) // xch
                x_tiles = {}

                def load_xchunk(ci):
                    if ci >= n_xchunks:
                        return
                    xt = xring.tile([128, 4, xch * BL], BF, tag="xchunk")
                    nc.sync.dma_start(
                        xt,
                        xT_d[:, ci * xch * BL:(ci + 1) * xch * BL]
                        .rearrange("(ko p) tb -> p ko tb", p=128),
                    )
                    x_tiles[ci] = xt

                load_xchunk(0)
                load_xchunk(1)

                g0_tiles = {}

                def emit_xg(t, stop):
                    """x-projection groups for step t into a fresh G0 tile."""
                    ci, tl = t // xch, t % xch
                    xt = x_tiles[ci]
                    g = pg0.tile([128, 512], FP, tag="g0")
                    g0_tiles[t] = g
                    for k in range(4):
                        lhsT = xt[:, k, tl * BL:(tl + 1) * BL]
                        for j in range(4):
                            mm(g[32 * j:32 * (j + 1), :], lhsT,
                               w0T_sb[:, k, 512 * j:512 * (j + 1)],
                               tp=(0, 32 * j),
                               start=(k == 0), stop=(stop and k == 3))

                emit_xg(0, stop=True)
                emit_xg(1, stop=False)
                emit_xg(2, stop=False)

                def nonlin(g, c_prev, lab):
                    """gates PSUM [128,512] -> (h_bf16, c_new). 3 ACT + 4-5 DVE."""
                    a = work.tile([128, 512], FP, tag=f"a_{lab}")
                    nc.scalar.activation(a[:, 0:384], g[:, 0:384], AF.Sigmoid)
                    nc.scalar.activation(a[:, 384:512], g[:, 384:512], AF.Tanh)
                    t1 = work.tile([128, 128], FP, tag=f"t1_{lab}")
                    if c_prev is not None:
                        t2 = work.tile([128, 128], FP, tag=f"t2_{lab}")
                        nc.gpsimd.tensor_mul(t2, a[:, 128:256], c_prev)
                    nc.vector.tensor_mul(t1, a[:, 0:128], a[:, 384:512])
                    if c_prev is None:
                        cn = t1
                    else:
                        cn = state.tile([128, 128], FP, tag=f"c_{lab}")
                        nc.vector.tensor_add(cn, t1, t2)
                    th = work.tile([128, 128], FP, tag=f"th_{lab}")
                    nc.scalar.activation(th, cn, AF.Tanh)
                    hb = work.tile([128, 128], BF, tag=f"h_{lab}")
                    nc.vector.tensor_mul(hb, a[:, 256:384], th)
                    return hb, cn

                def emit_transpose(hb, lab):
                    pt = ptrp.tile([128, 128], BF, tag=f"pt_{lab}")
                    nc.tensor.transpose(pt, hb, ident_sb)
                    ht = state.tile([128, 128], BF, tag=f"ht_{lab}")
                    nc.vector.tensor_copy(ht, pt)
                    return ht

                c0 = c1 = None
                h0b = h1b = None
                ht0 = ht1 = None
                for t in range(T):
                    # PE: finish gates0[t] (recurrent part)
                    if t >= 1:
                        g = g0_tiles[t]
                        for k in range(4):
                            lhsT = ht0[:, 32 * k:32 * (k + 1)]
                            for j in range(4):
                                mm(g[32 * j:32 * (j + 1), :], lhsT,
                                   wh0T_sb[:, k, 512 * j:512 * (j + 1)],
                                   tp=(0, 32 * j), start=False, stop=(k == 3))

                    # PE: xg for t+3 (independent filler, no PE wait)
                    if t + 3 < T:
                        emit_xg(t + 3, stop=False)

                    # PE: transpose h1[t-1]; DVE copy -> ht1
                    if t >= 1:
                        ht1 = emit_transpose(h1b, "l1")

                    # PE: gates1[t] bias + h1-recurrent part
                    g1 = pg1.tile([128, 512], FP, tag="g1")
                    for j in range(4):
                        mm(g1[32 * j:32 * (j + 1), :], ones_sb[:, :],
                           b1row_sb[:, 512 * j:512 * (j + 1)],
                           tp=(0, 32 * j), start=True, stop=False)
                    if t >= 1:
                        for k in range(4):
                            lhsT = ht1[:, 32 * k:32 * (k + 1)]
                            for j in range(4):
                                mm(g1[32 * j:32 * (j + 1), :], lhsT,
                                   w1T_sb[:, 4 + k, 512 * j:512 * (j + 1)],
                                   tp=(0, 32 * j), start=False, stop=False)

                    # n0[t]: ACT/DVE chain on G0[t]
                    h0b, c0 = nonlin(g0_tiles[t], c0, "l0")
                    g0_tiles.pop(t)

                    # PE: transpose h0[t]; DVE copy -> ht0
                    ht0 = emit_transpose(h0b, "l0")

                    # PE: gates1[t] h0-input part
                    for k in range(4):
                        lhsT = ht0[:, 32 * k:32 * (k + 1)]
                        for j in range(4):
                            mm(g1[32 * j:32 * (j + 1), :], lhsT,
                               w1T_sb[:, k, 512 * j:512 * (j + 1)],
                               tp=(0, 32 * j), start=False, stop=(k == 3))

                    # n1[t]
                    h1b, c1 = nonlin(g1, c1, "l1")

                    # DMA: stream x chunks and prefetch fc weights
                    if t % xch == 0 and t > 0:
                        load_xchunk(t // xch + 1)
                    if t >= 2 and t % 2 == 0 and (t - 2) // 2 < N_PREF:
                        i = (t - 2) // 2
                        nc.sync.dma_start(
                            fcw_pre[:, i, :, :],
                            fcwT_d[:, i * 512:(i + 1) * 512]
                            .rearrange("(ko p) n -> p ko n", p=128),
                        )

                # epilogue: final h1 transpose -> hlast
                pt = ptrp.tile([128, 128], BF, tag="pt_l1")
                nc.tensor.transpose(pt, h1b, ident_sb)
                nc.vector.tensor_copy(hlast, pt)

            # ---------------- phase 2: FC + Lambda layout + Sigma ----------------
            with (
                tc.tile_pool(name="fcstream", bufs=8) as fcsp,
                tc.tile_pool(name="fcb2", bufs=2) as fcb2p,
                tc.tile_pool(name="lt", bufs=1) as ltp,
                tc.tile_pool(name="sigw", bufs=4) as sigw,
                tc.tile_pool(name="pfc", bufs=2, space="PSUM") as pfcp,
                tc.tile_pool(name="psig", bufs=2, space="PSUM") as psigp,
            ):
                fcw_str = {}

                def stream_fcw(jj):
                    if jj < N_PREF or jj >= N_FTILES or jj in fcw_str:
                        return
                    ft = fcsp.tile([128, 4, 512], BF, tag="fcs", name=f"fcs{jj}")
                    nc.sync.dma_start(
                        ft,
                        fcwT_d[:, jj * 512:(jj + 1) * 512]
                        .rearrange("(ko p) n -> p ko n", p=128),
                    )
                    fcw_str[jj] = ft

                fcb_tiles = {}
                for q in range(2):
                    fq = fcb2p.tile([1, 2048], BF, tag="fcbq")
                    nc.sync.dma_start(fq, fcbrow_d[:, q * 2048:(q + 1) * 2048])
                    fcb_tiles[q] = fq
                for jj in range(N_PREF, N_PREF + 8):
                    stream_fcw(jj)

                LT = ltp.tile([32, 500, 32], FP)       # [factor, asset, b]
                F_sb = ltp.tile([32, 32], FP)          # exp(0.5*fvar raw) [factor, b]
                Fraw = ltp.tile([32, 32], FP)
                idio1_sb = ltp.tile([128, 384], FP)    # rows 96:128 used
                idio2_sb = ltp.tile([32, 116], FP)

                n_quads = (N_FTILES + 3) // 4          # 9 (last quad has 1 tile)
                for q in range(n_quads):
                    rr = range(4) if q < 8 else range(1)
                    if q in fcb_tiles:
                        fcb_q = fcb_tiles.pop(q)
                    else:
                        ncols = 2048 if q < 8 else 512
                        fcb_q = fcb2p.tile([1, 2048], BF, tag="fcbq")
                        nc.sync.dma_start(fcb_q[:, 0:ncols],
                                          fcbrow_d[:, q * 2048:q * 2048 + ncols])
                    pfc = pfcp.tile([128, 512], FP, tag="pfc")
                    for jn in range(4 * (q + 1), 4 * (q + 2)):
                        stream_fcw(jn)
                    for r in rr:
                        jj = 4 * q + r
                        fsrc = (fcw_pre[:, jj, :, :] if jj < N_PREF
                                else fcw_str[jj])
                        mm(pfc[32 * r:32 * (r + 1), :], ones_sb[:, :],
                           fcb_q[:, 512 * r:512 * r + 512],
                           tp=(0, 32 * r), start=True, stop=False)
                        for k in range(4):
                            mm(pfc[32 * r:32 * (r + 1), :],
                               hlast[:, 32 * k:32 * (k + 1)],
                               fsrc[:, k, :],
                               tp=(0, 32 * r), start=False, stop=(k == 3))

                    # Lambda blocks -> LT via DVE stream-transpose (32x32)
                    for r in rr:
                        jj = 4 * q + r
                        sl = slice(32 * r, 32 * (r + 1))
                        if jj < 31:
                            a0 = jj * 16
                            nc.vector.transpose(
                                LT[:, a0:a0 + 16, :],
                                pfc[sl, :].rearrange("p (qq f) -> p qq f", f=32),
                            )
                        elif jj == 31:
                            # Lambda tail: assets 496:500 (cols 0:128)
                            nc.vector.transpose(
                                LT[:, 496:500, :],
                                pfc[96:128, 0:128]
                                .rearrange("p (qq f) -> p qq f", f=32),
                            )
                            # idio[0:384] raw (cols 128:512, parts 96:128)
                            nc.scalar.copy(idio1_sb[96:128, :],
                                           pfc[96:128, 128:512])
                            nc.sync.dma_start(idio_d[:, 0:384],
                                              idio1_sb[96:128, :])
                        else:  # jj == 32
                            # fvar [b, f] cols 0:32 -> transpose -> exp -> [f, b]
                            nc.vector.transpose(Fraw, pfc[0:32, 0:32])
                            nc.scalar.activation(F_sb, Fraw, AF.Exp, scale=0.5)
                            nc.scalar.copy(idio2_sb, pfc[0:32, 32:148])
                            nc.sync.dma_start(idio_d[:, 384:500], idio2_sb)

                # Sigma per sample
                for b in range(BL):
                    gt = sigw.tile([32, 512], BF, tag="gt")
                    nc.vector.tensor_scalar_mul(gt[:, 0:500], LT[:, :, b],
                                                F_sb[:, b:b + 1])
                    for mt in range(4):
                        rows = 128 if mt < 3 else 116
                        ps = psigp.tile([128, 512], FP, tag="psig")
                        mm(ps[:rows, 0:500], gt[:, 128 * mt:128 * mt + rows],
                           gt[:, 0:500], tp=(0, 0), start=True, stop=True)
                        st = sigw.tile([128, 512], FP, tag="sigstage")
                        if mt % 2 == 0:
                            nc.scalar.copy(st[:rows, 0:500], ps[:rows, 0:500])
                        else:
                            nc.vector.tensor_copy(st[:rows, 0:500],
                                                  ps[:rows, 0:500])
                        nc.sync.dma_start(
                            sigma_d[b, 128 * mt:128 * mt + rows, :],
                            st[:rows, 0:500])

    nc.compile()
    return nc


# ---------------------------------------------------------------- entry point

def kernel(**inputs):
    from concourse.bass_utils import run_bass_kernel_spmd

    prep = host_prep_shared(inputs)
    x = np.asarray(inputs["x"], np.float32)
    in_maps = []
    for core in range(NCORES):
        m = dict(prep)
        m["xT"] = host_prep_x(x[core * BL:(core + 1) * BL])
        in_maps.append(m)

    nc = build_nc()
    res = run_bass_kernel_spmd(nc, in_maps, list(range(NCORES)))
    results = res.results

    idx = np.arange(NA)
    out = np.empty((B_FULL, NA, NA), np.float32)
    for core in range(NCORES):
        sigma = np.asarray(results[core]["sigma"]).astype(np.float32)
        idio = np.exp(np.asarray(results[core]["idio_raw"], np.float32))
        sigma[:, idx, idx] += idio
        out[core * BL:(core + 1) * BL] = sigma
    return out
